# revision 18
# baseline (speedup 1.0000x reference)
"""MoE layer kernel for Trainium2, 8 NeuronCores — remote-DMA v3.

Math (see reference): top-2-of-softmax routing reduces to dense routing
weights w[b,s,e]; since the output sums over tokens,
    z[b,e,i]  = sum_s w[b,s,e] * x[b,s,i]
    out[b,o]  = sum_e z[b,e,:] @ expert_w[e,o,:] + (sum_s w[b,s,e]) eb[e,o]
Core b computes gating + z for batch b; core e holds expert e's weights;
the host sums the per-core partial outputs.

v3 replaces the CC-stream AllToAll with direct SBUF-to-SBUF
remote_dma_broadcast sends. The NEFF then contains no collectives at
all, which removes the ~21.7us CC-core boot + 35-47us mesh barrier +
~11us first-op latency that put a hard ~70us floor under the collective
version.

Transport details:
  - Each wave (1024 tokens), each core transposes its z block to
    [128, 8chunk, 8expert] fp16, appends a rank-identity column, and
    sends the whole 2KB-per-partition payload to each of the 7 peers
    (XOR slot k -> physical tpb ^ k, relative addressing so one SPMD
    program works on every core), plus a local copy into slot 0.
  - The rank -> physical-core mapping need not be known: each receiver
    selects its own expert's row with a one-hot input (reduce over the
    expert axis), and the identity column rides into a tiny extra
    output so the HOST routes each partial-output row to the right
    batch. Self-send always lands in slot 0, so the (own-batch) bias
    is added to row 0 unconditionally.
  - The cross-core data-arrival wait (remote_sem >= 14 per wave) is
    injected into the scheduled stream after TileContext exits: the
    tile scheduler's single-core sim cannot model remote increments
    and would misreport a deadlock.

Precision: gate logits use bf16 x (high) + fp8e4m3 scaled residual
(low) planes against a split-bf16 gate weight stack -> logit error
~2e-4 keeps top-2 flips rare; z and expert matmuls run bf16 with
split-bf16 routing weights; z transport in fp16. Measured rel err
~5.4e-3 (harness gate 2e-2).
"""

import sys

import numpy as np

for _p in ("/opt/trn_rl_repo",):
    if _p not in sys.path:
        sys.path.insert(0, _p)

import ml_dtypes
import concourse.bass as bass
import concourse.mybir as mybir
from concourse import bacc
import concourse.tile as tile
from concourse.masks import make_identity

F32 = mybir.dt.float32
F16 = mybir.dt.float16
BF = mybir.dt.bfloat16
F8 = mybir.dt.float8e4
BF_NP = ml_dtypes.bfloat16
F8_NP = ml_dtypes.float8_e4m3
P = 128
NCORES = 8
B = 8
E = 8
I = 1024
O = 1024
IC = I // P          # 8 contraction chunks
NW = 2               # token waves
BIG = 1.0e30
CSCALE = 4096.0      # fp8 correction plane scale
PAY = IC * E + 1     # remote payload free size: 64 z values + identity
RSEM_PER_WAVE = 14   # 7 remote senders x 2 increments


def _inject_wait_before(nc, anchor_bass_inst, sem, value, engine):
    """Insert an EventSemaphore wait right before the (already scheduled)
    anchor instruction. Done post-TileContext because the scheduler's
    single-core sim cannot see remote semaphore increments."""
    wait_inst = engine.wait_ge(sem, value)
    raw = wait_inst.ins
    target = anchor_bass_inst.ins
    for blk in nc.main_func.blocks:
        if raw in blk.instructions:
            blk.instructions.remove(raw)
            break
    for blk in nc.main_func.blocks:
        if target in blk.instructions:
            idx = blk.instructions.index(target)
            blk.instructions.insert(idx, raw)
            return
    raise RuntimeError("anchor instruction not found in any block")


def build_nc(T: int = 2048):
    TW = T // NW                 # tokens per wave
    NTW = TW // P                # token tiles per wave
    nc = bacc.Bacc(num_devices=NCORES)

    xth_d = nc.dram_tensor("xth", [I, T], BF, kind="ExternalInput")
    xl8_d = nc.dram_tensor("xl8", [I, T], F8, kind="ExternalInput")
    xh_d = nc.dram_tensor("xh", [T, I], BF, kind="ExternalInput")
    gct_d = nc.dram_tensor("gct", [I, 40], BF, kind="ExternalInput")
    gc8_d = nc.dram_tensor("gc8", [I, 40], BF, kind="ExternalInput")
    gbc_d = nc.dram_tensor("gbc", [E, 1], F32, kind="ExternalInput")
    wth_d = nc.dram_tensor("wth", [I, O], BF, kind="ExternalInput")
    ebal_d = nc.dram_tensor("ebal", [E, O], BF, kind="ExternalInput")
    rnk_d = nc.dram_tensor("rnk", [P, 1], F16, kind="ExternalInput")
    esel_d = nc.dram_tensor("esel", [P, E], F16, kind="ExternalInput")
    out_d = nc.dram_tensor("out_p", [B, O], F32, kind="ExternalOutput")
    oid_d = nc.dram_tensor("oid", [P, E], F32, kind="ExternalOutput")

    rsem = nc.alloc_semaphore("rsem")
    lsem = nc.alloc_semaphore("lsem")
    anchors = []

    with tile.TileContext(nc) as tc:
        with (
            tc.tile_pool(name="singles", bufs=1) as singles,
            tc.tile_pool(name="wv", bufs=2) as wv,
            tc.tile_pool(name="ps_lgt", bufs=1, space="PSUM") as ps_lgt,
            tc.tile_pool(name="ps_ltp", bufs=1, space="PSUM") as ps_ltp,
            tc.tile_pool(name="ps_z", bufs=1, space="PSUM") as ps_z,
            tc.tile_pool(name="ps_zt", bufs=1, space="PSUM") as ps_zt,
            tc.tile_pool(name="ps_out", bufs=1, space="PSUM") as ps_out,
        ):
            # ---- tiny constants ----
            ident8 = singles.tile([E, E], F32)
            make_identity(nc, ident8)
            ident8h = singles.tile([E, E], F16)
            make_identity(nc, ident8h)
            ones_f = singles.tile([P, 1], F32)
            nc.gpsimd.memset(ones_f, 1.0)
            gct_sb = singles.tile([P, IC, 40], BF)
            nc.sync.dma_start(
                out=gct_sb, in_=gct_d[:].rearrange("(c p) g -> p c g", p=P)
            )
            gc8_sb = singles.tile([P, IC, 40], BF)
            nc.sync.dma_start(
                out=gc8_sb, in_=gc8_d[:].rearrange("(c p) g -> p c g", p=P)
            )
            gbc_sb = singles.tile([E, 1], F32)
            nc.sync.dma_start(out=gbc_sb, in_=gbc_d[:])
            ebal_sb = singles.tile([E, O], BF)
            nc.sync.dma_start(out=ebal_sb, in_=ebal_d[:])
            rnk_sb = singles.tile([P, 1], F16)
            nc.sync.dma_start(out=rnk_sb, in_=rnk_d[:])
            esel_sb = singles.tile([P, E], F16)
            nc.sync.dma_start(out=esel_sb, in_=esel_d[:])

            # receive buffer: [p, wave, slot, payload]
            zz_recv = singles.tile([P, NW, E, PAY], F16)

            # ---- bulk loads, wave-priority order; queues drain FIFO so
            # earlier dma_starts complete first ----
            xth_sb = singles.tile([P, IC, T], BF)
            xl8_sb = singles.tile([P, IC, T], F8)
            xh_sb = singles.tile([P, T // P, I], BF)
            wth_sb = singles.tile([P, IC, O], BF)
            xthv = xth_d[:].rearrange("(c p) t -> p c t", p=P)
            xl8v = xl8_d[:].rearrange("(c p) t -> p c t", p=P)
            xhv = xh_d[:].rearrange("(t p) i -> p t i", p=P)
            for w in range(NW):
                ws = slice(w * TW, (w + 1) * TW)
                for h in range(2):
                    cs = slice(h * (IC // 2), (h + 1) * (IC // 2))
                    nc.sync.dma_start(
                        out=xth_sb[:, cs, ws], in_=xthv[:, cs, ws]
                    )
                for h in range(2):
                    cs = slice(h * (IC // 2), (h + 1) * (IC // 2))
                    nc.sync.dma_start(
                        out=xl8_sb[:, cs, ws], in_=xl8v[:, cs, ws]
                    )
                ts = slice(w * NTW, (w + 1) * NTW)
                nc.sync.dma_start(out=xh_sb[:, ts, :], in_=xhv[:, ts, :])
            for h in range(2):
                cs = slice(h * (IC // 2), (h + 1) * (IC // 2))
                nc.sync.dma_start(out=wth_sb[:, cs, :], in_=wth_d[:].rearrange(
                    "(c p) o -> p c o", p=P)[:, cs, :])

            wsum_acc = singles.tile([P, E], F32)

            # ================= wave-pipelined gating + z =================
            # Emission order interleaves the waves so the PE never idles
            # through a DVE-bound routing phase:
            #   PE:  gate0, transp0, gate1, z0+send0, transp1, z1+send1
            #   DVE: evac0, top2_0, evac1, zevac0, top2_1, zevac1

            def gate_phase(w):
                lgt = ps_lgt.tile([40, TW], F32, tag="lgt", name="lgt")
                for c in range(IC):
                    for g in range(2):
                        gs = slice(w * TW + g * 512, w * TW + (g + 1) * 512)
                        go = slice(g * 512, (g + 1) * 512)
                        nc.tensor.matmul(
                            lgt[:, go],
                            gct_sb[:, c, :],
                            xth_sb[:, c, gs],
                            start=(c == 0),
                            stop=False,
                        )
                for c in range(IC):
                    for g in range(2):
                        gs = slice(w * TW + g * 512, w * TW + (g + 1) * 512)
                        go = slice(g * 512, (g + 1) * 512)
                        nc.tensor.matmul(
                            lgt[:, go],
                            gc8_sb[:, c, :],
                            xl8_sb[:, c, gs],
                            start=False,
                            stop=(c == IC - 1),
                        )
                # evacuate: l2s = gh-rows + gl-rows + gate_b (one PSUM
                # input per DVE op allowed)
                l2s = wv.tile([E, TW], F32, tag="l2s", name="l2s")
                nc.vector.tensor_scalar(
                    l2s, lgt[0:E, :], gbc_sb, None, mybir.AluOpType.add
                )
                nc.vector.tensor_add(l2s, l2s, lgt[32 : 32 + E, :])
                return l2s

            def route_phase(w, l2s):
                # transpose to token-major [tok, 8] tiles
                ltp = ps_ltp.tile([P, NTW, E], F32, tag="ltp", name="ltp")
                for t in range(NTW):
                    nc.tensor.transpose(
                        ltp[:, t, :], l2s[:, t * P : (t + 1) * P], ident8
                    )
                # top-2 -> dense routing weights (ltp read in place)
                m1 = wv.tile([P, NTW], F32, tag="m1", name="m1")
                nc.vector.reduce_max(m1, ltp, axis=mybir.AxisListType.X)
                is1 = wv.tile([P, NTW, E], F32, tag="is1", name="is1")
                nc.vector.tensor_tensor(
                    is1, ltp, m1[:, :, None].to_broadcast((P, NTW, E)),
                    mybir.AluOpType.is_ge,
                )
                msk = wv.tile([P, NTW, E], F32, tag="msk", name="msk")
                nc.vector.tensor_scalar(
                    msk, is1, BIG, None, mybir.AluOpType.mult
                )
                lm = wv.tile([P, NTW, E], F32, tag="lm", name="lm")
                nc.vector.tensor_sub(lm, ltp, msk)
                m2 = wv.tile([P, NTW], F32, tag="m2", name="m2")
                nc.vector.reduce_max(m2, lm, axis=mybir.AxisListType.X)
                is2 = wv.tile([P, NTW, E], F32, tag="is2", name="is2")
                nc.vector.tensor_tensor(
                    is2, lm, m2[:, :, None].to_broadcast((P, NTW, E)),
                    mybir.AluOpType.is_ge,
                )
                d12 = wv.tile([P, NTW], F32, tag="d12", name="d12")
                nc.vector.tensor_sub(d12, m2, m1)
                w2 = wv.tile([P, NTW], F32, tag="w2", name="w2")
                nc.scalar.activation(
                    w2, d12, mybir.ActivationFunctionType.Sigmoid
                )
                # wd = is1 + w2*(is2 - is1)
                t1 = wv.tile([P, NTW, E], F32, tag="t1", name="t1")
                nc.vector.tensor_sub(t1, is2, is1)
                nc.vector.tensor_tensor(
                    t1, t1, w2[:, :, None].to_broadcast((P, NTW, E)),
                    mybir.AluOpType.mult,
                )
                wd = wv.tile([P, NTW, E], F32, tag="wd", name="wd")
                nc.vector.tensor_add(wd, is1, t1)
                # wsum accumulation (strided view: sum over token tiles)
                if w == 0:
                    nc.vector.reduce_sum(
                        wsum_acc, wd[:].rearrange("p t e -> p e t"),
                        axis=mybir.AxisListType.X,
                    )
                else:
                    wsw = wv.tile([P, E], F32, tag="wsw", name="wsw")
                    nc.vector.reduce_sum(
                        wsw, wd[:].rearrange("p t e -> p e t"),
                        axis=mybir.AxisListType.X,
                    )
                    nc.vector.tensor_add(wsum_acc, wsum_acc, wsw)
                # split-bf16 routing weights [wh | wl]
                wc = wv.tile([P, NTW, 40], BF, tag="wc", name="wc")
                nc.vector.memset(wc, 0.0)
                nc.vector.tensor_copy(wc[:, :, 0:E], wd)
                nc.vector.tensor_tensor(
                    wc[:, :, 32:40], wd, wc[:, :, 0:E],
                    mybir.AluOpType.subtract,
                )
                return wc

            def z_phase(w, wc):
                z_ps = ps_z.tile([40, I], F32, tag="z", name="z_ps")
                for t in range(NTW):
                    for g in range(2):
                        go = slice(g * 512, (g + 1) * 512)
                        nc.tensor.matmul(
                            z_ps[:, go],
                            wc[:, t, :],
                            xh_sb[:, w * NTW + t, go],
                            start=(t == 0),
                            stop=(t == NTW - 1),
                        )
                zf = wv.tile([E, I], F32, tag="zf", name="zf")
                nc.vector.tensor_copy(zf, z_ps[0:E, :])
                z_h = wv.tile([E, I], F16, tag="zh", name="z_h")
                nc.vector.tensor_tensor(
                    z_h, zf, z_ps[32 : 32 + E, :], mybir.AluOpType.add
                )
                # transpose z to [p, chunk, expert] and build the payload
                zt = ps_zt.tile([P, IC, E], F16, tag="zt", name="zt")
                for c in range(IC):
                    nc.tensor.transpose(
                        zt[:, c, :], z_h[:, c * P : (c + 1) * P], ident8h
                    )
                zsend = wv.tile([P, PAY], F16, tag="zsend", name="zsend")
                nc.vector.tensor_copy(
                    zsend[:, 0 : IC * E],
                    zt[:].rearrange("p c e -> p (c e)"),
                )
                nc.vector.tensor_copy(zsend[:, IC * E : PAY], rnk_sb)
                # local copy into slot 0; remote sends to the 7 peers
                nc.vector.tensor_copy(zz_recv[:, w, 0, :], zsend)
                for k in range(1, E):
                    rdests = [None] * E
                    rdests[k] = (0, k)
                    nc.gpsimd.remote_dma_broadcast(
                        zz_recv[:, w, k, :], zsend[:], rsem, lsem,
                        rdests=rdests,
                    )
                nc.gpsimd.trigger_dma(count=None)

            l2s0 = gate_phase(0)
            wc0 = route_phase(0, l2s0)
            l2s1 = gate_phase(1)
            z_phase(0, wc0)
            wc1 = route_phase(1, l2s1)
            z_phase(1, wc1)

            # ================= expert matmul =================
            out_ps = ps_out.tile([E, O], F32)
            for w in range(NW):
                # select my expert's row from each received block:
                # zzc[p,c,slot] = sum_e zz[p,w,slot,c*8+e] * onehot[e]
                blkv = zz_recv[:, w, :, 0 : IC * E].rearrange(
                    "p k (c e) -> p c k e", e=E
                )
                sel = wv.tile([P, IC, E, E], F16, tag="sel", name="sel")
                a = nc.vector.tensor_tensor(
                    sel, blkv,
                    esel_sb[:, None, None, :].to_broadcast((P, IC, E, E)),
                    mybir.AluOpType.mult,
                )
                anchors.append(a)
                zzc = wv.tile([P, IC, E], F16, tag="zzc", name="zzc")
                with nc.allow_low_precision(
                    reason="one-hot select: single nonzero term, exact"
                ):
                    nc.vector.reduce_sum(
                        zzc, sel, axis=mybir.AxisListType.X
                    )
                for c in range(IC):
                    for g in range(2):
                        go = slice(g * 512, (g + 1) * 512)
                        nc.tensor.matmul(
                            out_ps[:, go],
                            zzc[:, c, :],
                            wth_sb[:, c, go],
                            start=(w == 0 and c == 0),
                            stop=False,
                        )
                if w == 0:
                    # identity column -> host routing info
                    oid_sb = singles.tile([P, E], F32)
                    nc.vector.tensor_copy(oid_sb, zz_recv[:, 0, :, IC * E])
                    nc.sync.dma_start(out=oid_d[:], in_=oid_sb)
                    # wsum -> bias stationary (own batch = slot 0)
                    ws_ps = ps_ltp.tile([P, NTW, E], F32, tag="ltp",
                                        name="ws_ps")
                    nc.tensor.matmul(
                        ws_ps[0:E, 0, 0:1], wsum_acc, ones_f,
                        start=True, stop=True,
                    )
                    wsum8 = singles.tile([E, E], BF)
                    nc.vector.memset(wsum8, 0.0)
                    nc.vector.tensor_copy(
                        wsum8[:, 0:1], ws_ps[0:E, 0, 0:1]
                    )
            # bias: out_ps[0,:] += sum_e wsum[e] * ebal[e,:]
            for g in range(2):
                go = slice(g * 512, (g + 1) * 512)
                nc.tensor.matmul(
                    out_ps[:, go],
                    wsum8,
                    ebal_sb[:, go],
                    start=False,
                    stop=(g == 1),
                )
            out_sb = singles.tile([B, O], F32)
            for h in range(2):
                hs = slice(h * 512, (h + 1) * 512)
                nc.vector.tensor_copy(out_sb[:, hs], out_ps[:, hs])
                nc.sync.dma_start(out=out_d[:, hs], in_=out_sb[:, hs])

    for w in range(NW):
        _inject_wait_before(
            nc, anchors[w], rsem, RSEM_PER_WAVE * (w + 1), nc.vector
        )
    nc.compile()
    return nc


_NC_CACHE: dict = {}


def _get_nc(T: int):
    if T not in _NC_CACHE:
        _NC_CACHE[T] = build_nc(T)
    return _NC_CACHE[T]


def make_in_maps(x, gate_w, gate_b, expert_w, expert_b):
    x = np.ascontiguousarray(np.asarray(x, dtype=np.float32))
    gw = np.asarray(gate_w, dtype=np.float32)
    gb = np.asarray(gate_b, dtype=np.float32)
    ew = np.asarray(expert_w, dtype=np.float32)
    eb = np.asarray(expert_b, dtype=np.float32)
    _, S, _ = x.shape

    gwt = np.ascontiguousarray(gw.T)                   # [I, E]
    gh = gwt.astype(BF_NP)
    gl = (gwt - gh.astype(np.float32)).astype(BF_NP)
    gct = np.zeros((I, 40), BF_NP)
    gct[:, 0:E] = gh
    gct[:, 32:40] = gl
    gc8 = np.zeros((I, 40), BF_NP)
    gc8[:, 0:E] = (gwt / CSCALE).astype(BF_NP)
    gbc = np.ascontiguousarray(gb.reshape(E, 1))
    ebal = eb.astype(BF_NP)                            # [E, O]

    in_maps = []
    for c in range(NCORES):
        xc = x[c]                                      # [S, I]
        xch = xc.astype(BF_NP)
        xl = xc - xch.astype(np.float32)
        xth = np.ascontiguousarray(xch.T)              # [I, S]
        xl8 = np.ascontiguousarray((xl.T * CSCALE).astype(F8_NP))
        wth = np.ascontiguousarray(ew[c].T.astype(BF_NP))  # [I, O]
        esel = np.zeros((P, E), np.float16)
        esel[:, c] = 1.0
        in_maps.append(
            {
                "xth": xth,
                "xl8": xl8,
                "xh": np.ascontiguousarray(xch),
                "gct": gct,
                "gc8": gc8,
                "gbc": gbc,
                "wth": wth,
                "ebal": np.ascontiguousarray(ebal),
                "rnk": np.full((P, 1), float(c), np.float16),
                "esel": esel,
            }
        )
    return in_maps, S


def _run(inputs: dict, trace: bool = False):
    from concourse.bass_utils import run_bass_kernel_spmd

    in_maps, S = make_in_maps(
        inputs["x"],
        inputs["gate_w"],
        inputs["gate_b"],
        inputs["expert_w"],
        inputs["expert_b"],
    )
    nc = _get_nc(S)
    res = run_bass_kernel_spmd(
        nc, in_maps, core_ids=list(range(NCORES)), trace=trace
    )
    out = np.zeros((B, O), np.float64)
    for c in range(NCORES):
        ids = res.results[c]["oid"][0]                 # [8] sender ranks
        idx = np.rint(ids).astype(int)
        assert sorted(idx.tolist()) == list(range(B)), (
            f"core {c}: bad identity column {ids}"
        )
        part = res.results[c]["out_p"].astype(np.float64)
        for k in range(B):
            out[idx[k]] += part[k]
    return out.astype(np.float32), res


def kernel(**inputs) -> np.ndarray:
    out, _ = _run(inputs, trace=False)
    return out


def run_traced(**inputs):
    out, res = _run(inputs, trace=True)
    return out, res
